# revision 6
# baseline (speedup 1.0000x reference)
"""Trainium2 Bass kernel for masked-softmax attention (sparse_attention).

reference:
    S = Q @ K^T / sqrt(128)            # [N, nq, nk]
    A = softmax(S, axis=-1) * mask
    A = A / (sum_k A + 1e-6)
    O = A @ V

Device identity (softmax normalizer cancels in the renormalization):
    E = exp(S); P = E * mask
    O[q, :] = (P @ V)[q, :] / sum_k P[q, k]
(the reference's +1e-6 is ~2e-6 relative to the masked sum and dropped).

Sharding: N=32 batch-heads split across 8 NeuronCores, 4 per core; no
cross-core communication.

The elementwise chain exp+mask is the throughput wall (ACT does 1
elem/lane/cycle @1.2GHz -> ~109us/core for 16.8M exps), so 2 of every
16 k-tiles compute P on the DVE+GPSIMD instead via a Schraudolph
fast-exp with the mask folded in:
    K^T is pre-scaled by A/sqrt(d) (A = 2^23/ln2), so mm1 yields
    s' = A*S.  Host supplies lnmB = 127*2^23 - C + A*ln(mask); then
    int32(s' + lnmB) reinterpreted as f32 is exp(S)*mask to ~1.8% rms
    (C=486000 minimizes rms).  DVE does the add+int32 convert, GPSIMD
    does the bitcast->bf16 copy.  The remaining 14 tiles use ACT exp
    (activation scale 1/A restores the true exponent) and paired DVE
    multiplies (one [128,2048] 2x op per two k-tiles).

Per-core pipeline over 8 slabs (batch b, q-half h of 1024):
  slab prologue: prefetch NEXT slab's mask + lnmB (big contiguous DMAs)
    and, at h==0, next batch's K^T/Q^T/V, so DMA runs a slab ahead.
  k-phase per k-tile kt: mm1 (PE, bf16) -> s' in PSUM; then ACT exp ->
    DVE mask-mult (pairs), or DVE add+i32 -> GPSIMD copy (kt 6,7).
  q-phase (interleaved one q-tile per 2 kt, on the PREVIOUS slab's P^T):
    mm2 (PE, bf16): O|denom = sum_kt PT[kt][:,qc].T @ [V_kt | 1] -> PSUM
    recip+scale (DVE): st[qc] = O * (1/denom)  [bf16]
  store st -> out per slab (8 q-tiles, sync HWDGE ring).
"""
import sys

sys.path.insert(0, "/opt/trn_rl_repo")

import ml_dtypes
import numpy as np

from concourse import bacc, mybir, tile
from concourse.bass_utils import run_bass_kernel_spmd

N, NQ, NK, D = 32, 2048, 2048, 128
N_CORES = 8
B = N // N_CORES          # batches per core
KT = NK // 128            # k tiles per batch
QT = NQ // 128            # q tiles per batch
QTH = QT // 2             # q tiles per slab
QH = NQ // 2              # q-half width
SCALE = float(1.0 / np.sqrt(D))

# Schraudolph fast-exp constants (see module docstring)
EXP_A = float(2.0 ** 23 / np.log(2.0))
EXP_C = 486000.0
OFF = (6, 7)              # k-tiles computed via fast-exp on DVE+GPSIMD
KEEP = [kt for kt in range(KT) if kt not in OFF]   # ACT-exp tiles, paired
NKEEP = len(KEEP)

F32 = mybir.dt.float32
I32 = mybir.dt.int32
BF16 = mybir.dt.bfloat16

_cached = {}


def build():
    if "nc" in _cached:
        return _cached["nc"]
    nc = bacc.Bacc("TRN2", target_bir_lowering=False, debug=False)

    qt_d = nc.dram_tensor("queriesT", [B, 2, D, QH], BF16, kind="ExternalInput").ap()
    kt_d = nc.dram_tensor("keysT", [B, D, NK], BF16, kind="ExternalInput").ap()
    v_d = nc.dram_tensor("valuesP", [B, 128, KT, D + 1], BF16, kind="ExternalInput").ap()
    m_d = nc.dram_tensor("maskT", [B, 2, 128, NKEEP, QH], BF16, kind="ExternalInput").ap()
    ln_d = nc.dram_tensor("lnmB", [B, 2, 128, len(OFF), QH], F32, kind="ExternalInput").ap()
    o_d = nc.dram_tensor("out", [B, 2, 128, QTH, D], BF16, kind="ExternalOutput").ap()

    with tile.TileContext(nc) as tc:
        with (
            tc.tile_pool(name="tr", bufs=2) as trpool,
            tc.tile_pool(name="qth", bufs=4) as qpool,
            tc.tile_pool(name="vbo", bufs=2) as vpool,
            tc.tile_pool(name="maskt", bufs=2) as mpool,
            tc.tile_pool(name="work", bufs=3) as wpool,
            tc.tile_pool(name="ti32", bufs=2) as tipool,
            tc.tile_pool(name="ptslab", bufs=2) as ptpool,
            tc.tile_pool(name="stage", bufs=4) as stpool,
            tc.tile_pool(name="spsum", bufs=2, space="PSUM") as spool,
            tc.tile_pool(name="opsum", bufs=4, space="PSUM") as opool,
        ):
            def issue_mask(i):
                b, h = divmod(i, 2)
                mlo = mpool.tile([128, 8, QH], BF16, tag="mlo")
                mhi = mpool.tile([128, NKEEP - 8, QH], BF16, tag="mhi")
                lnm = mpool.tile([128, len(OFF), QH], F32, tag="lnm")
                nc.sync.dma_start(mlo[:], m_d[b, h, :, 0:8, :])
                nc.sync.dma_start(lnm[:], ln_d[b, h])
                nc.sync.dma_start(mhi[:], m_d[b, h, :, 8:NKEEP, :])
                return mlo, mhi, lnm

            def issue_batch(bb):
                kt_a = trpool.tile([128, 512], BF16, tag="kta")
                kt_b = trpool.tile([128, NK - 512], BF16, tag="ktb")
                qt_h0 = qpool.tile([128, QH], BF16, tag="qt")
                qt_h1 = qpool.tile([128, QH], BF16, tag="qt")
                vb = vpool.tile([128, KT, D + 1], BF16, tag="vb")
                nc.sync.dma_start(kt_a[:], kt_d[bb, :, 0:512])
                nc.sync.dma_start(qt_h0[:], qt_d[bb, 0])
                nc.sync.dma_start(kt_b[:], kt_d[bb, :, 512:NK])
                nc.sync.dma_start(qt_h1[:], qt_d[bb, 1])
                nc.sync.dma_start(vb[:], v_d[bb])
                return kt_a, kt_b, (qt_h0, qt_h1), vb

            def issue_batch0():
                # batch 0: ordered so mm1 never starves; masks overlap exps
                kt_a = trpool.tile([128, 512], BF16, tag="kta")
                kt_b = trpool.tile([128, NK - 512], BF16, tag="ktb")
                qt_h0 = qpool.tile([128, QH], BF16, tag="qt")
                qt_h1 = qpool.tile([128, QH], BF16, tag="qt")
                vb = vpool.tile([128, KT, D + 1], BF16, tag="vb")
                mlo = mpool.tile([128, 8, QH], BF16, tag="mlo")
                mhi = mpool.tile([128, NKEEP - 8, QH], BF16, tag="mhi")
                lnm = mpool.tile([128, len(OFF), QH], F32, tag="lnm")
                nc.sync.dma_start(kt_a[:], kt_d[0, :, 0:512])
                nc.sync.dma_start(qt_h0[:], qt_d[0, 0])
                nc.sync.dma_start(kt_b[:], kt_d[0, :, 512:NK])
                nc.sync.dma_start(mlo[:], m_d[0, 0, :, 0:8, :])
                nc.sync.dma_start(lnm[:], ln_d[0, 0])
                nc.sync.dma_start(mhi[:], m_d[0, 0, :, 8:NKEEP, :])
                nc.sync.dma_start(qt_h1[:], qt_d[0, 1])
                nc.sync.dma_start(vb[:], v_d[0])
                return kt_a, kt_b, (qt_h0, qt_h1), vb, (mlo, mhi, lnm)

            def q_iter(prev, qc):
                """One q-tile of the q-phase for a finished P^T slab."""
                pt, vb, st, b, h = prev
                o_ps = opool.tile([128, D + 1], F32, tag="o")
                for kt in range(KT):
                    nc.tensor.matmul(
                        o_ps[:],
                        pt[:, kt, qc * 128:(qc + 1) * 128],
                        vb[:, kt, :],
                        start=(kt == 0),
                        stop=(kt == KT - 1),
                    )
                rd = wpool.tile([128, 1], F32, tag="rd")
                nc.vector.reciprocal(rd[:], o_ps[:, D:D + 1])
                nc.vector.tensor_scalar_mul(st[:, qc, :], o_ps[:, 0:D], rd[:])
                if qc == QTH - 1:
                    nc.sync.dma_start(o_d[b, h], st[:])

            kt_a, kt_b, qt_hs, vb, mask_cur = issue_batch0()

            prev = None
            for i in range(2 * B):
                b, h = divmod(i, 2)
                # prefetch next slab's mask, then next batch's K/Q/V
                mask_next = issue_mask(i + 1) if i + 1 < 2 * B else None
                if h == 0 and b + 1 < B:
                    nxt = issue_batch(b + 1)
                st = stpool.tile([128, QTH, D], BF16, tag="st")

                qt_sb = qt_hs[h]
                mlo, mhi, lnm = mask_cur
                pt = ptpool.tile([128, KT, QH], BF16, tag="pt")
                e_pair = None
                n_seen = 0   # ACT-exp tiles emitted so far (indexes m_d)
                for kt in range(KT):
                    s_ps = spool.tile([128, QH], F32, tag="s")
                    ksrc = kt_a[:, kt * 128:(kt + 1) * 128] if kt < 4 else \
                        kt_b[:, (kt - 4) * 128:(kt - 3) * 128]
                    for c in range(2):
                        nc.tensor.matmul(
                            s_ps[:, c * 512:(c + 1) * 512],
                            ksrc,
                            qt_sb[:, c * 512:(c + 1) * 512],
                            start=True,
                            stop=True,
                        )
                    if kt in OFF:
                        # fast-exp with folded mask: bits(exp(S)*m) =
                        # int32(A*S + lnmB); bitcast, downconvert on GPSIMD
                        ti = tipool.tile([128, QH], I32, tag="ti")
                        nc.vector.tensor_tensor(
                            out=ti[:],
                            in0=s_ps[:],
                            in1=lnm[:, kt - OFF[0], :],
                            op=mybir.AluOpType.add,
                        )
                        nc.gpsimd.tensor_copy(pt[:, kt, :], ti[:].bitcast(F32))
                    else:
                        if n_seen % 2 == 0:
                            e_pair = wpool.tile([128, 2, QH], BF16, tag="e")
                        nc.scalar.activation(
                            e_pair[:, n_seen % 2, :],
                            s_ps[:],
                            mybir.ActivationFunctionType.Exp,
                            scale=1.0 / EXP_A,
                        )
                        if n_seen % 2 == 1:
                            # paired 2x multiply over two k-tiles at once
                            j = n_seen - 1
                            msrc = mlo[:, j:j + 2, :] if j < 8 else \
                                mhi[:, j - 8:j - 6, :]
                            kt0 = KEEP[j]
                            nc.vector.tensor_tensor(
                                out=pt[:, kt0:kt0 + 2, :],
                                in0=e_pair[:],
                                in1=msrc,
                                op=mybir.AluOpType.mult,
                            )
                        n_seen += 1
                    # interleave the previous slab's q-phase into this
                    # k-phase (one q-tile per two k-tiles)
                    if prev is not None and kt % 2 == 0:
                        q_iter(prev, kt // 2)
                prev = (pt, vb, st, b, h)
                mask_cur = mask_next
                if h == 1 and b + 1 < B:
                    kt_a, kt_b, qt_hs, vb = nxt

            for qc in range(QTH):
                q_iter(prev, qc)

    nc.compile()
    _cached["nc"] = nc
    return nc


def kernel(queries, keys, values, mask, _trace=False, **kw):
    queries = np.asarray(queries, dtype=np.float32)
    keys = np.asarray(keys, dtype=np.float32)
    values = np.asarray(values, dtype=np.float32)
    mask = np.asarray(mask, dtype=np.float32)
    nc = build()
    bf16 = ml_dtypes.bfloat16
    ktiles = np.array(KEEP)
    otiles = np.array(OFF)
    lnoff = np.float64(127 * 2 ** 23) - np.float64(EXP_C)
    in_maps = []
    for c in range(N_CORES):
        sl = slice(c * B, (c + 1) * B)
        vv = values[sl].reshape(B, KT, 128, D).transpose(0, 2, 1, 3)
        v_aug = np.concatenate(
            [vv, np.ones((B, 128, KT, 1), np.float32)], axis=3
        )
        # mask as [B, k, q] -> [B, kt, 128, 2(h), QH] -> [B, 2, 128, kt, QH]
        mt = (
            mask[sl]
            .transpose(0, 2, 1)
            .reshape(B, KT, 128, 2, QH)
            .transpose(0, 3, 2, 1, 4)
        )
        lnmB = (
            lnoff
            + np.float64(EXP_A)
            * np.clip(np.log(np.maximum(mt[:, :, :, otiles, :], 1e-38),
                             dtype=np.float64), -60.0, 0.0)
        ).astype(np.float32)
        in_maps.append(
            {
                "queriesT": np.ascontiguousarray(
                    queries[sl].transpose(0, 2, 1).reshape(B, D, 2, QH)
                    .transpose(0, 2, 1, 3)
                ).astype(bf16),
                "keysT": np.ascontiguousarray(
                    keys[sl].transpose(0, 2, 1) * np.float32(EXP_A * SCALE)
                ).astype(bf16),
                "valuesP": np.ascontiguousarray(v_aug).astype(bf16),
                "maskT": np.ascontiguousarray(
                    mt[:, :, :, ktiles, :]
                ).astype(bf16),
                "lnmB": np.ascontiguousarray(lnmB),
            }
        )
    res = run_bass_kernel_spmd(
        nc, in_maps, core_ids=list(range(N_CORES)), trace=_trace
    )
    out = np.concatenate(
        [
            res.results[c]["out"]
            .astype(np.float32)
            .transpose(0, 1, 3, 2, 4)
            .reshape(B, NQ, D)
            for c in range(N_CORES)
        ],
        axis=0,
    )
    if _trace:
        return out, res
    return out


# revision 7
# speedup vs baseline: 1.2343x; 1.2343x over previous
"""Trainium2 Bass kernel for masked-softmax attention (sparse_attention).

reference:
    S = Q @ K^T / sqrt(128)            # [N, nq, nk]
    A = softmax(S, axis=-1) * mask
    A = A / (sum_k A + 1e-6)
    O = A @ V

Device identity (softmax normalizer cancels in the renormalization):
    E = exp(S); P = E * mask
    O[q, :] = (P @ V)[q, :] / sum_k P[q, k]
(the reference's +1e-6 is ~2e-6 relative to the masked sum and dropped).

Sharding: N=32 batch-heads split across 8 NeuronCores, 4 per core; no
cross-core communication.

The elementwise chain exp+mask is the throughput wall (ACT does 1
elem/lane/cycle @1.2GHz -> ~109us/core for 16.8M exps), so 2 of every
16 k-tiles compute P on the DVE+GPSIMD instead via a Schraudolph
fast-exp with the mask folded in:
    K^T is pre-scaled by A/sqrt(d) (A = 2^23/ln2), so mm1 yields
    s' = A*S.  Host supplies lnmB = 127*2^23 - C + A*ln(mask); then
    int32(s' + lnmB) reinterpreted as f32 is exp(S)*mask to ~1.8% rms
    (C=486000 minimizes rms).  DVE does the add+int32 convert, GPSIMD
    does the bitcast->bf16 copy.  The remaining 14 tiles use ACT exp
    (activation scale 1/A restores the true exponent) and paired DVE
    multiplies (one [128,2048] 2x op per two k-tiles).

Per-core pipeline over 8 slabs (batch b, q-half h of 1024):
  slab prologue: prefetch NEXT slab's mask + lnmB (big contiguous DMAs)
    and, at h==0, next batch's K^T/Q^T/V, so DMA runs a slab ahead.
  k-phase per k-tile kt: mm1 (PE, bf16) -> s' in PSUM; then ACT exp ->
    DVE mask-mult (pairs), or DVE add+i32 -> GPSIMD copy (kt 6,7).
  q-phase (interleaved one q-tile per 2 kt, on the PREVIOUS slab's P^T):
    mm2 (PE, bf16): O|denom = sum_kt PT[kt][:,qc].T @ [V_kt | 1] -> PSUM
    recip+scale (DVE): st[qc] = O * (1/denom)  [bf16]
  store st -> out per slab (8 q-tiles, sync HWDGE ring).
"""
import sys

sys.path.insert(0, "/opt/trn_rl_repo")

import ml_dtypes
import numpy as np

from concourse import bacc, mybir, tile
from concourse.bass_utils import run_bass_kernel_spmd

N, NQ, NK, D = 32, 2048, 2048, 128
N_CORES = 8
B = N // N_CORES          # batches per core
KT = NK // 128            # k tiles per batch
QT = NQ // 128            # q tiles per batch
QTH = QT // 2             # q tiles per slab
QH = NQ // 2              # q-half width
SCALE = float(1.0 / np.sqrt(D))

# Schraudolph fast-exp constants, in bf16 bit-space (see module docstring)
EXP_A = float(2.0 ** 7 / np.log(2.0))
EXP_C = float(486000.0 / 2 ** 16)
OFF = (6, 7)              # k-tiles computed via fast-exp on DVE+GPSIMD
KEEP = [kt for kt in range(KT) if kt not in OFF]   # ACT-exp tiles, paired
NKEEP = len(KEEP)

F32 = mybir.dt.float32
I16 = mybir.dt.int16
BF16 = mybir.dt.bfloat16

_cached = {}


def build():
    if "nc" in _cached:
        return _cached["nc"]
    nc = bacc.Bacc("TRN2", target_bir_lowering=False, debug=False)

    qt_d = nc.dram_tensor("queriesT", [B, 2, D, QH], BF16, kind="ExternalInput").ap()
    kt_d = nc.dram_tensor("keysT", [B, D, NK], BF16, kind="ExternalInput").ap()
    v_d = nc.dram_tensor("valuesP", [B, 128, KT, D + 1], BF16, kind="ExternalInput").ap()
    m_d = nc.dram_tensor("maskT", [B, 2, 128, NKEEP, QH], BF16, kind="ExternalInput").ap()
    ln_d = nc.dram_tensor("lnmB", [B, 2, 128, len(OFF), QH], F32, kind="ExternalInput").ap()
    o_d = nc.dram_tensor("out", [B, 2, 128, QTH, D], BF16, kind="ExternalOutput").ap()

    with tile.TileContext(nc) as tc:
        with (
            tc.tile_pool(name="tr", bufs=2) as trpool,
            tc.tile_pool(name="qth", bufs=4) as qpool,
            tc.tile_pool(name="vbo", bufs=2) as vpool,
            tc.tile_pool(name="maskt", bufs=2) as mpool,
            tc.tile_pool(name="work", bufs=3) as wpool,
            tc.tile_pool(name="ptslab", bufs=2) as ptpool,
            tc.tile_pool(name="stage", bufs=4) as stpool,
            tc.tile_pool(name="spsum", bufs=2, space="PSUM") as spool,
            tc.tile_pool(name="opsum", bufs=4, space="PSUM") as opool,
        ):
            def issue_mask(i):
                b, h = divmod(i, 2)
                mlo = mpool.tile([128, 8, QH], BF16, tag="mlo")
                mhi = mpool.tile([128, NKEEP - 8, QH], BF16, tag="mhi")
                lnm = mpool.tile([128, len(OFF), QH], F32, tag="lnm")
                nc.sync.dma_start(mlo[:], m_d[b, h, :, 0:8, :])
                nc.sync.dma_start(lnm[:], ln_d[b, h])
                nc.sync.dma_start(mhi[:], m_d[b, h, :, 8:NKEEP, :])
                return mlo, mhi, lnm

            def issue_batch(bb):
                kt_a = trpool.tile([128, 512], BF16, tag="kta")
                kt_b = trpool.tile([128, NK - 512], BF16, tag="ktb")
                qt_h0 = qpool.tile([128, QH], BF16, tag="qt")
                qt_h1 = qpool.tile([128, QH], BF16, tag="qt")
                vb = vpool.tile([128, KT, D + 1], BF16, tag="vb")
                nc.sync.dma_start(kt_a[:], kt_d[bb, :, 0:512])
                nc.sync.dma_start(qt_h0[:], qt_d[bb, 0])
                nc.sync.dma_start(kt_b[:], kt_d[bb, :, 512:NK])
                nc.sync.dma_start(qt_h1[:], qt_d[bb, 1])
                nc.sync.dma_start(vb[:], v_d[bb])
                return kt_a, kt_b, (qt_h0, qt_h1), vb

            def issue_batch0():
                # batch 0: ordered so mm1 never starves; masks overlap exps
                kt_a = trpool.tile([128, 512], BF16, tag="kta")
                kt_b = trpool.tile([128, NK - 512], BF16, tag="ktb")
                qt_h0 = qpool.tile([128, QH], BF16, tag="qt")
                qt_h1 = qpool.tile([128, QH], BF16, tag="qt")
                vb = vpool.tile([128, KT, D + 1], BF16, tag="vb")
                mlo = mpool.tile([128, 8, QH], BF16, tag="mlo")
                mhi = mpool.tile([128, NKEEP - 8, QH], BF16, tag="mhi")
                lnm = mpool.tile([128, len(OFF), QH], F32, tag="lnm")
                nc.sync.dma_start(kt_a[:], kt_d[0, :, 0:512])
                nc.sync.dma_start(qt_h0[:], qt_d[0, 0])
                nc.sync.dma_start(kt_b[:], kt_d[0, :, 512:NK])
                nc.sync.dma_start(mlo[:], m_d[0, 0, :, 0:8, :])
                nc.sync.dma_start(lnm[:], ln_d[0, 0])
                nc.sync.dma_start(mhi[:], m_d[0, 0, :, 8:NKEEP, :])
                nc.sync.dma_start(qt_h1[:], qt_d[0, 1])
                nc.sync.dma_start(vb[:], v_d[0])
                return kt_a, kt_b, (qt_h0, qt_h1), vb, (mlo, mhi, lnm)

            def q_iter(prev, qc):
                """One q-tile of the q-phase for a finished P^T slab."""
                pt, vb, st, b, h = prev
                o_ps = opool.tile([128, D + 1], F32, tag="o")
                for kt in range(KT):
                    nc.tensor.matmul(
                        o_ps[:],
                        pt[:, kt, qc * 128:(qc + 1) * 128],
                        vb[:, kt, :],
                        start=(kt == 0),
                        stop=(kt == KT - 1),
                    )
                rd = wpool.tile([128, 1], F32, tag="rd")
                nc.vector.reciprocal(rd[:], o_ps[:, D:D + 1])
                nc.vector.tensor_scalar_mul(st[:, qc, :], o_ps[:, 0:D], rd[:])
                if qc == QTH - 1:
                    nc.sync.dma_start(o_d[b, h], st[:])

            kt_a, kt_b, qt_hs, vb, mask_cur = issue_batch0()

            prev = None
            for i in range(2 * B):
                b, h = divmod(i, 2)
                # prefetch next slab's mask, then next batch's K/Q/V
                mask_next = issue_mask(i + 1) if i + 1 < 2 * B else None
                if h == 0 and b + 1 < B:
                    nxt = issue_batch(b + 1)
                st = stpool.tile([128, QTH, D], BF16, tag="st")

                qt_sb = qt_hs[h]
                mlo, mhi, lnm = mask_cur
                pt = ptpool.tile([128, KT, QH], BF16, tag="pt")
                e_pair = None
                n_seen = 0   # ACT-exp tiles emitted so far (indexes m_d)
                for kt in range(KT):
                    s_ps = spool.tile([128, QH], F32, tag="s")
                    ksrc = kt_a[:, kt * 128:(kt + 1) * 128] if kt < 4 else \
                        kt_b[:, (kt - 4) * 128:(kt - 3) * 128]
                    for c in range(2):
                        nc.tensor.matmul(
                            s_ps[:, c * 512:(c + 1) * 512],
                            ksrc,
                            qt_sb[:, c * 512:(c + 1) * 512],
                            start=True,
                            stop=True,
                        )
                    if kt in OFF:
                        # fast-exp with folded mask, straight to bf16 bits:
                        # bf16_bits(exp(S)*m) = int16(A*S + lnmB)
                        nc.vector.tensor_tensor(
                            out=pt[:, kt, :].bitcast(I16),
                            in0=s_ps[:],
                            in1=lnm[:, kt - OFF[0], :],
                            op=mybir.AluOpType.add,
                        )
                    else:
                        if n_seen % 2 == 0:
                            e_pair = wpool.tile([128, 2, QH], BF16, tag="e")
                        nc.scalar.activation(
                            e_pair[:, n_seen % 2, :],
                            s_ps[:],
                            mybir.ActivationFunctionType.Exp,
                            scale=1.0 / EXP_A,
                        )
                        if n_seen % 2 == 1:
                            # paired 2x multiply over two k-tiles at once
                            j = n_seen - 1
                            msrc = mlo[:, j:j + 2, :] if j < 8 else \
                                mhi[:, j - 8:j - 6, :]
                            kt0 = KEEP[j]
                            nc.vector.tensor_tensor(
                                out=pt[:, kt0:kt0 + 2, :],
                                in0=e_pair[:],
                                in1=msrc,
                                op=mybir.AluOpType.mult,
                            )
                        n_seen += 1
                    # interleave the previous slab's q-phase into this
                    # k-phase (one q-tile per two k-tiles)
                    if prev is not None and kt % 2 == 0:
                        q_iter(prev, kt // 2)
                prev = (pt, vb, st, b, h)
                mask_cur = mask_next
                if h == 1 and b + 1 < B:
                    kt_a, kt_b, qt_hs, vb = nxt

            for qc in range(QTH):
                q_iter(prev, qc)

    nc.compile()
    _cached["nc"] = nc
    return nc


def kernel(queries, keys, values, mask, _trace=False, **kw):
    queries = np.asarray(queries, dtype=np.float32)
    keys = np.asarray(keys, dtype=np.float32)
    values = np.asarray(values, dtype=np.float32)
    mask = np.asarray(mask, dtype=np.float32)
    nc = build()
    bf16 = ml_dtypes.bfloat16
    ktiles = np.array(KEEP)
    otiles = np.array(OFF)
    lnoff = np.float64(127 * 2 ** 7) - np.float64(EXP_C)
    in_maps = []
    for c in range(N_CORES):
        sl = slice(c * B, (c + 1) * B)
        vv = values[sl].reshape(B, KT, 128, D).transpose(0, 2, 1, 3)
        v_aug = np.concatenate(
            [vv, np.ones((B, 128, KT, 1), np.float32)], axis=3
        )
        # mask as [B, k, q] -> [B, kt, 128, 2(h), QH] -> [B, 2, 128, kt, QH]
        mt = (
            mask[sl]
            .transpose(0, 2, 1)
            .reshape(B, KT, 128, 2, QH)
            .transpose(0, 3, 2, 1, 4)
        )
        lnmB = (
            lnoff
            + np.float64(EXP_A)
            * np.clip(np.log(np.maximum(mt[:, :, :, otiles, :], 1e-38),
                             dtype=np.float64), -60.0, 0.0)
        ).astype(np.float32)
        in_maps.append(
            {
                "queriesT": np.ascontiguousarray(
                    queries[sl].transpose(0, 2, 1).reshape(B, D, 2, QH)
                    .transpose(0, 2, 1, 3)
                ).astype(bf16),
                "keysT": np.ascontiguousarray(
                    keys[sl].transpose(0, 2, 1) * np.float32(EXP_A * SCALE)
                ).astype(bf16),
                "valuesP": np.ascontiguousarray(v_aug).astype(bf16),
                "maskT": np.ascontiguousarray(
                    mt[:, :, :, ktiles, :]
                ).astype(bf16),
                "lnmB": np.ascontiguousarray(lnmB),
            }
        )
    res = run_bass_kernel_spmd(
        nc, in_maps, core_ids=list(range(N_CORES)), trace=_trace
    )
    out = np.concatenate(
        [
            res.results[c]["out"]
            .astype(np.float32)
            .transpose(0, 1, 3, 2, 4)
            .reshape(B, NQ, D)
            for c in range(N_CORES)
        ],
        axis=0,
    )
    if _trace:
        return out, res
    return out


# revision 8
# speedup vs baseline: 1.2762x; 1.0339x over previous
"""Trainium2 Bass kernel for masked-softmax attention (sparse_attention).

reference:
    S = Q @ K^T / sqrt(128)            # [N, nq, nk]
    A = softmax(S, axis=-1) * mask
    A = A / (sum_k A + 1e-6)
    O = A @ V

Device identity (softmax normalizer cancels in the renormalization):
    E = exp(S); P = E * mask
    O[q, :] = (P @ V)[q, :] / sum_k P[q, k]
(the reference's +1e-6 is ~2e-6 relative to the masked sum and dropped).

Sharding: N=32 batch-heads split across 8 NeuronCores, 4 per core; no
cross-core communication.

The elementwise chain exp+mask is the throughput wall (ACT does 1
elem/lane/cycle @1.2GHz -> ~109us/core for 16.8M exps), so 3 of every
16 k-tiles compute P on the DVE instead, via a Schraudolph fast-exp in
bf16 bit-space with the mask folded in:
    K^T is pre-scaled by A/sqrt(d) with A = 2^7/ln2, so mm1 yields
    s' = A*S.  The host supplies lnmB = int16(127*2^7 - C + A*ln(mask));
    then int16(s' + lnmB), reinterpreted as bf16, IS exp(S)*mask to
    ~1.8% rms (C minimizes rms; the linear-in-mantissa interpolation is
    the only error source).  One DVE tensor_tensor(add) per tile, with
    the int16 convert-on-write landing directly in the P^T slab.
    The remaining 13 tiles use ACT exp (activation scale 1/A restores
    the true exponent) and paired DVE multiplies (one [128,2048] 2x op
    per two k-tiles).

Per-core pipeline over 8 slabs (batch b, q-half h of 1024):
  slab prologue: prefetch NEXT slab's mask + lnmB (big contiguous DMAs)
    and, at h==0, next batch's K^T/Q^T/V, so DMA runs a slab ahead.
  k-phase per k-tile kt: mm1 (PE, bf16) -> s' in PSUM; then ACT exp ->
    DVE mask-mult (pairs), or the DVE fast-exp (kt 6,7,14).
  q-phase (interleaved one q-tile per 2 kt, on the PREVIOUS slab's P^T):
    mm2 (PE, bf16): O|denom = sum_kt PT[kt][:,qc].T @ [V_kt | 1] -> PSUM
    recip+scale (DVE): st[qc] = O * (1/denom)  [bf16]
  store st -> out per slab (8 q-tiles, sync HWDGE ring).
"""
import sys

sys.path.insert(0, "/opt/trn_rl_repo")

import ml_dtypes
import numpy as np

from concourse import bacc, mybir, tile
from concourse.bass_utils import run_bass_kernel_spmd

N, NQ, NK, D = 32, 2048, 2048, 128
N_CORES = 8
B = N // N_CORES          # batches per core
KT = NK // 128            # k tiles per batch
QT = NQ // 128            # q tiles per batch
QTH = QT // 2             # q tiles per slab
QH = NQ // 2              # q-half width
SCALE = float(1.0 / np.sqrt(D))

# Schraudolph fast-exp constants, in bf16 bit-space (see module docstring)
EXP_A = float(2.0 ** 7 / np.log(2.0))
EXP_C = float(486000.0 / 2 ** 16)
OFF = (6, 7, 14)          # k-tiles computed via the DVE fast-exp
KEEP = [kt for kt in range(KT) if kt not in OFF]   # ACT-exp tiles
NKEEP = len(KEEP)
NLO = 8                   # KEEP tiles in the low mask tile

F32 = mybir.dt.float32
I16 = mybir.dt.int16
BF16 = mybir.dt.bfloat16

_cached = {}


def build():
    if "nc" in _cached:
        return _cached["nc"]
    nc = bacc.Bacc("TRN2", target_bir_lowering=False, debug=False)

    qt_d = nc.dram_tensor("queriesT", [B, 2, D, QH], BF16, kind="ExternalInput").ap()
    kt_d = nc.dram_tensor("keysT", [B, D, NK], BF16, kind="ExternalInput").ap()
    v_d = nc.dram_tensor("valuesP", [B, 128, KT, D + 1], BF16, kind="ExternalInput").ap()
    m_d = nc.dram_tensor("maskT", [B, 2, 128, NKEEP, QH], BF16, kind="ExternalInput").ap()
    ln_d = nc.dram_tensor("lnmB", [B, 2, 128, len(OFF), QH], I16, kind="ExternalInput").ap()
    o_d = nc.dram_tensor("out", [B, 2, 128, QTH, D], BF16, kind="ExternalOutput").ap()

    with tile.TileContext(nc) as tc:
        with (
            tc.tile_pool(name="tr", bufs=2) as trpool,
            tc.tile_pool(name="qth", bufs=4) as qpool,
            tc.tile_pool(name="vbo", bufs=2) as vpool,
            tc.tile_pool(name="maskt", bufs=2) as mpool,
            tc.tile_pool(name="work", bufs=4) as wpool,
            tc.tile_pool(name="ptslab", bufs=2) as ptpool,
            tc.tile_pool(name="stage", bufs=4) as stpool,
            tc.tile_pool(name="spsum", bufs=3, space="PSUM") as spool,
            tc.tile_pool(name="opsum", bufs=2, space="PSUM") as opool,
        ):
            def issue_mask(i):
                b, h = divmod(i, 2)
                mlo = mpool.tile([128, NLO, QH], BF16, tag="mlo")
                mhi = mpool.tile([128, NKEEP - NLO, QH], BF16, tag="mhi")
                lnm = mpool.tile([128, len(OFF), QH], I16, tag="lnm")
                nc.sync.dma_start(mlo[:], m_d[b, h, :, 0:NLO, :])
                nc.sync.dma_start(lnm[:], ln_d[b, h])
                nc.sync.dma_start(mhi[:], m_d[b, h, :, NLO:NKEEP, :])
                return mlo, mhi, lnm

            def issue_batch(bb):
                kt_a = trpool.tile([128, 512], BF16, tag="kta")
                kt_b = trpool.tile([128, NK - 512], BF16, tag="ktb")
                qt_h0 = qpool.tile([128, QH], BF16, tag="qt")
                qt_h1 = qpool.tile([128, QH], BF16, tag="qt")
                vb = vpool.tile([128, KT, D + 1], BF16, tag="vb")
                nc.sync.dma_start(kt_a[:], kt_d[bb, :, 0:512])
                nc.sync.dma_start(qt_h0[:], qt_d[bb, 0])
                nc.sync.dma_start(kt_b[:], kt_d[bb, :, 512:NK])
                nc.sync.dma_start(qt_h1[:], qt_d[bb, 1])
                nc.sync.dma_start(vb[:], v_d[bb])
                return kt_a, kt_b, (qt_h0, qt_h1), vb

            def issue_batch0():
                # batch 0: FIFO order tuned so nothing downstream starves:
                # mm1(kt0..3) <- kt_a+qt_h0; first mults <- mlo; then kt_b
                # for mm1(kt4+), lnm/mhi for the later tiles, h=1 inputs last
                kt_a = trpool.tile([128, 512], BF16, tag="kta")
                kt_b = trpool.tile([128, NK - 512], BF16, tag="ktb")
                qt_h0 = qpool.tile([128, QH], BF16, tag="qt")
                qt_h1 = qpool.tile([128, QH], BF16, tag="qt")
                vb = vpool.tile([128, KT, D + 1], BF16, tag="vb")
                mlo = mpool.tile([128, NLO, QH], BF16, tag="mlo")
                mhi = mpool.tile([128, NKEEP - NLO, QH], BF16, tag="mhi")
                lnm = mpool.tile([128, len(OFF), QH], I16, tag="lnm")
                nc.sync.dma_start(kt_a[:], kt_d[0, :, 0:512])
                nc.sync.dma_start(qt_h0[:], qt_d[0, 0])
                nc.sync.dma_start(mlo[:], m_d[0, 0, :, 0:NLO, :])
                nc.sync.dma_start(kt_b[:], kt_d[0, :, 512:NK])
                nc.sync.dma_start(lnm[:], ln_d[0, 0])
                nc.sync.dma_start(mhi[:], m_d[0, 0, :, NLO:NKEEP, :])
                nc.sync.dma_start(qt_h1[:], qt_d[0, 1])
                nc.sync.dma_start(vb[:], v_d[0])
                return kt_a, kt_b, (qt_h0, qt_h1), vb, (mlo, mhi, lnm)

            def q_iter(prev, qc):
                """One q-tile of the q-phase for a finished P^T slab."""
                pt, vb, st, b, h = prev
                o_ps = opool.tile([128, D + 1], F32, tag="o")
                for kt in range(KT):
                    nc.tensor.matmul(
                        o_ps[:],
                        pt[:, kt, qc * 128:(qc + 1) * 128],
                        vb[:, kt, :],
                        start=(kt == 0),
                        stop=(kt == KT - 1),
                    )
                rd = wpool.tile([128, 1], F32, tag="rd")
                nc.vector.reciprocal(rd[:], o_ps[:, D:D + 1])
                nc.vector.tensor_scalar_mul(st[:, qc, :], o_ps[:, 0:D], rd[:])
                if qc == QTH - 1:
                    nc.sync.dma_start(o_d[b, h], st[:])

            kt_a, kt_b, qt_hs, vb, mask_cur = issue_batch0()

            prev = None
            for i in range(2 * B):
                b, h = divmod(i, 2)
                # prefetch next slab's mask, then next batch's K/Q/V
                mask_next = issue_mask(i + 1) if i + 1 < 2 * B else None
                if h == 0 and b + 1 < B:
                    nxt = issue_batch(b + 1)
                st = stpool.tile([128, QTH, D], BF16, tag="st")

                qt_sb = qt_hs[h]
                mlo, mhi, lnm = mask_cur
                pt = ptpool.tile([128, KT, QH], BF16, tag="pt")
                e_pair = None
                n_seen = 0   # ACT-exp tiles emitted so far (indexes m_d)
                for kt in range(KT):
                    s_ps = spool.tile([128, QH], F32, tag="s")
                    ksrc = kt_a[:, kt * 128:(kt + 1) * 128] if kt < 4 else \
                        kt_b[:, (kt - 4) * 128:(kt - 3) * 128]
                    for c in range(2):
                        nc.tensor.matmul(
                            s_ps[:, c * 512:(c + 1) * 512],
                            ksrc,
                            qt_sb[:, c * 512:(c + 1) * 512],
                            start=True,
                            stop=True,
                        )
                    if kt in OFF:
                        # fast-exp with folded mask, straight to bf16 bits:
                        # bf16_bits(exp(S)*m) = int16(A*S + lnmB)
                        nc.vector.tensor_tensor(
                            out=pt[:, kt, :].bitcast(I16),
                            in0=s_ps[:],
                            in1=lnm[:, OFF.index(kt), :],
                            op=mybir.AluOpType.add,
                        )
                    else:
                        if n_seen % 2 == 0:
                            e_pair = wpool.tile([128, 2, QH], BF16, tag="e")
                        nc.scalar.activation(
                            e_pair[:, n_seen % 2, :],
                            s_ps[:],
                            mybir.ActivationFunctionType.Exp,
                            scale=1.0 / EXP_A,
                        )
                        if n_seen % 2 == 1:
                            # paired 2x multiply over two k-tiles at once
                            j = n_seen - 1
                            msrc = mlo[:, j:j + 2, :] if j < NLO else \
                                mhi[:, j - NLO:j - NLO + 2, :]
                            kt0 = KEEP[j]
                            nc.vector.tensor_tensor(
                                out=pt[:, kt0:kt0 + 2, :],
                                in0=e_pair[:],
                                in1=msrc,
                                op=mybir.AluOpType.mult,
                            )
                        elif kt == KT - 1:
                            # odd KEEP count: single multiply for the last
                            j = n_seen
                            msrc = mhi[:, j - NLO, :]
                            nc.vector.tensor_tensor(
                                out=pt[:, kt, :],
                                in0=e_pair[:, 0, :],
                                in1=msrc,
                                op=mybir.AluOpType.mult,
                            )
                        n_seen += 1
                    # interleave the previous slab's q-phase into this
                    # k-phase (one q-tile per two k-tiles)
                    if prev is not None and kt % 2 == 0:
                        q_iter(prev, kt // 2)
                prev = (pt, vb, st, b, h)
                mask_cur = mask_next
                if h == 1 and b + 1 < B:
                    kt_a, kt_b, qt_hs, vb = nxt

            for qc in range(QTH):
                q_iter(prev, qc)

    nc.compile()
    _cached["nc"] = nc
    return nc


def kernel(queries, keys, values, mask, _trace=False, **kw):
    queries = np.asarray(queries, dtype=np.float32)
    keys = np.asarray(keys, dtype=np.float32)
    values = np.asarray(values, dtype=np.float32)
    mask = np.asarray(mask, dtype=np.float32)
    nc = build()
    bf16 = ml_dtypes.bfloat16
    ktiles = np.array(KEEP)
    otiles = np.array(OFF)
    lnoff = np.float64(127 * 2 ** 7) - np.float64(EXP_C)
    in_maps = []
    for c in range(N_CORES):
        sl = slice(c * B, (c + 1) * B)
        vv = values[sl].reshape(B, KT, 128, D).transpose(0, 2, 1, 3)
        v_aug = np.concatenate(
            [vv, np.ones((B, 128, KT, 1), np.float32)], axis=3
        )
        # mask as [B, k, q] -> [B, kt, 128, 2(h), QH] -> [B, 2, 128, kt, QH]
        mt = (
            mask[sl]
            .transpose(0, 2, 1)
            .reshape(B, KT, 128, 2, QH)
            .transpose(0, 3, 2, 1, 4)
        )
        lnmB = np.round(
            lnoff
            + np.float64(EXP_A)
            * np.clip(np.log(np.maximum(mt[:, :, :, otiles, :], 1e-38),
                             dtype=np.float64), -60.0, 0.0)
        ).astype(np.int16)
        in_maps.append(
            {
                "queriesT": np.ascontiguousarray(
                    queries[sl].transpose(0, 2, 1).reshape(B, D, 2, QH)
                    .transpose(0, 2, 1, 3)
                ).astype(bf16),
                "keysT": np.ascontiguousarray(
                    keys[sl].transpose(0, 2, 1) * np.float32(EXP_A * SCALE)
                ).astype(bf16),
                "valuesP": np.ascontiguousarray(v_aug).astype(bf16),
                "maskT": np.ascontiguousarray(
                    mt[:, :, :, ktiles, :]
                ).astype(bf16),
                "lnmB": np.ascontiguousarray(lnmB),
            }
        )
    res = run_bass_kernel_spmd(
        nc, in_maps, core_ids=list(range(N_CORES)), trace=_trace
    )
    out = np.concatenate(
        [
            res.results[c]["out"]
            .astype(np.float32)
            .transpose(0, 1, 3, 2, 4)
            .reshape(B, NQ, D)
            for c in range(N_CORES)
        ],
        axis=0,
    )
    if _trace:
        return out, res
    return out


# revision 9
# speedup vs baseline: 1.3386x; 1.0489x over previous
"""Trainium2 Bass kernel for masked-softmax attention (sparse_attention).

reference:
    S = Q @ K^T / sqrt(128)            # [N, nq, nk]
    A = softmax(S, axis=-1) * mask
    A = A / (sum_k A + 1e-6)
    O = A @ V

Device identity (softmax normalizer cancels in the renormalization):
    E = exp(S); P = E * mask
    O[q, :] = (P @ V)[q, :] / sum_k P[q, k]
(the reference's +1e-6 is ~2e-6 relative to the masked sum and dropped).

Sharding: N=32 batch-heads split across 8 NeuronCores, 4 per core; no
cross-core communication.

The elementwise chain exp+mask is the throughput wall (ACT does 1
elem/lane/cycle @1.2GHz -> ~109us/core for 16.8M exps), so 4 of every
16 k-tiles compute P on the DVE instead, via a Schraudolph fast-exp in
bf16 bit-space with the mask folded in:
    K^T is pre-scaled by A/sqrt(d) with A = 2^7/ln2, so mm1 yields
    s' = A*S.  The host supplies lnmB = int16(127*2^7 - C + A*ln(mask));
    then int16(s' + lnmB), reinterpreted as bf16, IS exp(S)*mask to
    ~1.8% rms (C minimizes rms; the linear-in-mantissa interpolation is
    the only error source).  One DVE tensor_tensor(add) per tile, with
    the int16 convert-on-write landing directly in the P^T slab.
    The remaining 12 tiles use ACT exp (activation scale 1/A restores
    the true exponent) and paired DVE multiplies (one [128,2048] 2x op
    per two k-tiles).

Per-core pipeline over 8 slabs (batch b, q-half h of 1024):
  slab prologue: prefetch NEXT slab's mask + lnmB (big contiguous DMAs)
    and, at h==0, next batch's K^T/Q^T/V, so DMA runs a slab ahead.
  k-phase per k-tile kt: mm1 (PE, bf16) -> s' in PSUM; then ACT exp ->
    DVE mask-mult (pairs), or the DVE fast-exp (kt 6,7,14).
  q-phase (interleaved one q-tile per 2 kt, on the PREVIOUS slab's P^T):
    mm2 (PE, bf16): O|denom = sum_kt PT[kt][:,qc].T @ [V_kt | 1] -> PSUM
    recip+scale (DVE): st[qc] = O * (1/denom)  [bf16]
  store st -> out per slab (8 q-tiles, sync HWDGE ring).
"""
import sys

sys.path.insert(0, "/opt/trn_rl_repo")

import ml_dtypes
import numpy as np

from concourse import bacc, mybir, tile
from concourse.bass_utils import run_bass_kernel_spmd

N, NQ, NK, D = 32, 2048, 2048, 128
N_CORES = 8
B = N // N_CORES          # batches per core
KT = NK // 128            # k tiles per batch
QT = NQ // 128            # q tiles per batch
QTH = QT // 2             # q tiles per slab
QH = NQ // 2              # q-half width
SCALE = float(1.0 / np.sqrt(D))

# Schraudolph fast-exp constants, in bf16 bit-space (see module docstring)
EXP_A = float(2.0 ** 7 / np.log(2.0))
EXP_C = float(486000.0 / 2 ** 16)
OFF = (6, 7, 14, 15)      # k-tiles computed via the DVE fast-exp
KEEP = [kt for kt in range(KT) if kt not in OFF]   # ACT-exp tiles
NKEEP = len(KEEP)
NLO = 8                   # KEEP tiles in the low mask tile

F32 = mybir.dt.float32
I16 = mybir.dt.int16
BF16 = mybir.dt.bfloat16

_cached = {}


def build():
    if "nc" in _cached:
        return _cached["nc"]
    nc = bacc.Bacc("TRN2", target_bir_lowering=False, debug=False)

    qt_d = nc.dram_tensor("queriesT", [B, 2, D, QH], BF16, kind="ExternalInput").ap()
    kt_d = nc.dram_tensor("keysT", [B, D, NK], BF16, kind="ExternalInput").ap()
    v_d = nc.dram_tensor("valuesP", [B, 128, KT, D + 1], BF16, kind="ExternalInput").ap()
    m_d = nc.dram_tensor("maskT", [B, 2, 128, NKEEP, QH], BF16, kind="ExternalInput").ap()
    ln_d = nc.dram_tensor("lnmB", [B, 2, 128, len(OFF), QH], I16, kind="ExternalInput").ap()
    o_d = nc.dram_tensor("out", [B, 2, 128, QTH, D], BF16, kind="ExternalOutput").ap()

    with tile.TileContext(nc) as tc:
        with (
            tc.tile_pool(name="tr", bufs=2) as trpool,
            tc.tile_pool(name="qth", bufs=4) as qpool,
            tc.tile_pool(name="vbo", bufs=2) as vpool,
            tc.tile_pool(name="maskt", bufs=2) as mpool,
            tc.tile_pool(name="work", bufs=4) as wpool,
            tc.tile_pool(name="ptslab", bufs=2) as ptpool,
            tc.tile_pool(name="stage", bufs=4) as stpool,
            tc.tile_pool(name="spsum", bufs=3, space="PSUM") as spool,
            tc.tile_pool(name="opsum", bufs=2, space="PSUM") as opool,
        ):
            def issue_mask(i):
                b, h = divmod(i, 2)
                mlo = mpool.tile([128, NLO, QH], BF16, tag="mlo")
                mhi = mpool.tile([128, NKEEP - NLO, QH], BF16, tag="mhi")
                lnm = mpool.tile([128, len(OFF), QH], I16, tag="lnm")
                nc.sync.dma_start(mlo[:], m_d[b, h, :, 0:NLO, :])
                nc.sync.dma_start(lnm[:], ln_d[b, h])
                nc.sync.dma_start(mhi[:], m_d[b, h, :, NLO:NKEEP, :])
                return mlo, mhi, lnm

            def issue_batch(bb):
                kt_a = trpool.tile([128, 512], BF16, tag="kta")
                kt_b = trpool.tile([128, NK - 512], BF16, tag="ktb")
                qt_h0 = qpool.tile([128, QH], BF16, tag="qt")
                qt_h1 = qpool.tile([128, QH], BF16, tag="qt")
                vb = vpool.tile([128, KT, D + 1], BF16, tag="vb")
                nc.sync.dma_start(kt_a[:], kt_d[bb, :, 0:512])
                nc.sync.dma_start(qt_h0[:], qt_d[bb, 0])
                nc.sync.dma_start(kt_b[:], kt_d[bb, :, 512:NK])
                nc.sync.dma_start(qt_h1[:], qt_d[bb, 1])
                nc.sync.dma_start(vb[:], v_d[bb])
                return kt_a, kt_b, (qt_h0, qt_h1), vb

            def issue_batch0():
                # batch 0: FIFO order tuned so nothing downstream starves:
                # mm1(kt0..3) <- kt_a+qt_h0; first mults <- mlo; then kt_b
                # for mm1(kt4+), lnm/mhi for the later tiles, h=1 inputs last
                kt_a = trpool.tile([128, 512], BF16, tag="kta")
                kt_b = trpool.tile([128, NK - 512], BF16, tag="ktb")
                qt_h0 = qpool.tile([128, QH], BF16, tag="qt")
                qt_h1 = qpool.tile([128, QH], BF16, tag="qt")
                vb = vpool.tile([128, KT, D + 1], BF16, tag="vb")
                mlo = mpool.tile([128, NLO, QH], BF16, tag="mlo")
                mhi = mpool.tile([128, NKEEP - NLO, QH], BF16, tag="mhi")
                lnm = mpool.tile([128, len(OFF), QH], I16, tag="lnm")
                nc.sync.dma_start(kt_a[:], kt_d[0, :, 0:512])
                nc.sync.dma_start(qt_h0[:], qt_d[0, 0])
                nc.sync.dma_start(mlo[:], m_d[0, 0, :, 0:NLO, :])
                nc.sync.dma_start(kt_b[:], kt_d[0, :, 512:NK])
                nc.sync.dma_start(lnm[:], ln_d[0, 0])
                nc.sync.dma_start(mhi[:], m_d[0, 0, :, NLO:NKEEP, :])
                nc.sync.dma_start(qt_h1[:], qt_d[0, 1])
                nc.sync.dma_start(vb[:], v_d[0])
                return kt_a, kt_b, (qt_h0, qt_h1), vb, (mlo, mhi, lnm)

            def q_iter(prev, qc):
                """One q-tile of the q-phase for a finished P^T slab."""
                pt, vb, st, b, h = prev
                o_ps = opool.tile([128, D + 1], F32, tag="o")
                for kt in range(KT):
                    nc.tensor.matmul(
                        o_ps[:],
                        pt[:, kt, qc * 128:(qc + 1) * 128],
                        vb[:, kt, :],
                        start=(kt == 0),
                        stop=(kt == KT - 1),
                    )
                rd = wpool.tile([128, 1], F32, tag="rd")
                nc.vector.reciprocal(rd[:], o_ps[:, D:D + 1])
                nc.vector.tensor_scalar_mul(st[:, qc, :], o_ps[:, 0:D], rd[:])
                if qc == QTH - 1:
                    nc.sync.dma_start(o_d[b, h], st[:])

            kt_a, kt_b, qt_hs, vb, mask_cur = issue_batch0()

            prev = None
            for i in range(2 * B):
                b, h = divmod(i, 2)
                # prefetch next slab's mask, then next batch's K/Q/V
                mask_next = issue_mask(i + 1) if i + 1 < 2 * B else None
                if h == 0 and b + 1 < B:
                    nxt = issue_batch(b + 1)
                st = stpool.tile([128, QTH, D], BF16, tag="st")

                qt_sb = qt_hs[h]
                mlo, mhi, lnm = mask_cur
                pt = ptpool.tile([128, KT, QH], BF16, tag="pt")
                e_pair = None
                n_seen = 0   # ACT-exp tiles emitted so far (indexes m_d)
                for kt in range(KT):
                    s_ps = spool.tile([128, QH], F32, tag="s")
                    ksrc = kt_a[:, kt * 128:(kt + 1) * 128] if kt < 4 else \
                        kt_b[:, (kt - 4) * 128:(kt - 3) * 128]
                    for c in range(2):
                        nc.tensor.matmul(
                            s_ps[:, c * 512:(c + 1) * 512],
                            ksrc,
                            qt_sb[:, c * 512:(c + 1) * 512],
                            start=True,
                            stop=True,
                        )
                    if kt in OFF:
                        # fast-exp with folded mask, straight to bf16 bits:
                        # bf16_bits(exp(S)*m) = int16(A*S + lnmB)
                        nc.vector.tensor_tensor(
                            out=pt[:, kt, :].bitcast(I16),
                            in0=s_ps[:],
                            in1=lnm[:, OFF.index(kt), :],
                            op=mybir.AluOpType.add,
                        )
                    else:
                        if n_seen % 2 == 0:
                            e_pair = wpool.tile([128, 2, QH], BF16, tag="e")
                        nc.scalar.activation(
                            e_pair[:, n_seen % 2, :],
                            s_ps[:],
                            mybir.ActivationFunctionType.Exp,
                            scale=1.0 / EXP_A,
                        )
                        if n_seen % 2 == 1:
                            # paired 2x multiply over two k-tiles at once
                            j = n_seen - 1
                            msrc = mlo[:, j:j + 2, :] if j < NLO else \
                                mhi[:, j - NLO:j - NLO + 2, :]
                            kt0 = KEEP[j]
                            nc.vector.tensor_tensor(
                                out=pt[:, kt0:kt0 + 2, :],
                                in0=e_pair[:],
                                in1=msrc,
                                op=mybir.AluOpType.mult,
                            )
                        elif kt == KT - 1:
                            # odd KEEP count: single multiply for the last
                            j = n_seen
                            msrc = mhi[:, j - NLO, :]
                            nc.vector.tensor_tensor(
                                out=pt[:, kt, :],
                                in0=e_pair[:, 0, :],
                                in1=msrc,
                                op=mybir.AluOpType.mult,
                            )
                        n_seen += 1
                    # interleave the previous slab's q-phase into this
                    # k-phase (one q-tile per two k-tiles)
                    if prev is not None and kt % 2 == 0:
                        q_iter(prev, kt // 2)
                prev = (pt, vb, st, b, h)
                mask_cur = mask_next
                if h == 1 and b + 1 < B:
                    kt_a, kt_b, qt_hs, vb = nxt

            for qc in range(QTH):
                q_iter(prev, qc)

    nc.compile()
    _cached["nc"] = nc
    return nc


def kernel(queries, keys, values, mask, _trace=False, **kw):
    queries = np.asarray(queries, dtype=np.float32)
    keys = np.asarray(keys, dtype=np.float32)
    values = np.asarray(values, dtype=np.float32)
    mask = np.asarray(mask, dtype=np.float32)
    nc = build()
    bf16 = ml_dtypes.bfloat16
    ktiles = np.array(KEEP)
    otiles = np.array(OFF)
    lnoff = np.float64(127 * 2 ** 7) - np.float64(EXP_C)
    in_maps = []
    for c in range(N_CORES):
        sl = slice(c * B, (c + 1) * B)
        vv = values[sl].reshape(B, KT, 128, D).transpose(0, 2, 1, 3)
        v_aug = np.concatenate(
            [vv, np.ones((B, 128, KT, 1), np.float32)], axis=3
        )
        # mask as [B, k, q] -> [B, kt, 128, 2(h), QH] -> [B, 2, 128, kt, QH]
        mt = (
            mask[sl]
            .transpose(0, 2, 1)
            .reshape(B, KT, 128, 2, QH)
            .transpose(0, 3, 2, 1, 4)
        )
        lnmB = np.round(
            lnoff
            + np.float64(EXP_A)
            * np.clip(np.log(np.maximum(mt[:, :, :, otiles, :], 1e-38),
                             dtype=np.float64), -60.0, 0.0)
        ).astype(np.int16)
        in_maps.append(
            {
                "queriesT": np.ascontiguousarray(
                    queries[sl].transpose(0, 2, 1).reshape(B, D, 2, QH)
                    .transpose(0, 2, 1, 3)
                ).astype(bf16),
                "keysT": np.ascontiguousarray(
                    keys[sl].transpose(0, 2, 1) * np.float32(EXP_A * SCALE)
                ).astype(bf16),
                "valuesP": np.ascontiguousarray(v_aug).astype(bf16),
                "maskT": np.ascontiguousarray(
                    mt[:, :, :, ktiles, :]
                ).astype(bf16),
                "lnmB": np.ascontiguousarray(lnmB),
            }
        )
    res = run_bass_kernel_spmd(
        nc, in_maps, core_ids=list(range(N_CORES)), trace=_trace
    )
    out = np.concatenate(
        [
            res.results[c]["out"]
            .astype(np.float32)
            .transpose(0, 1, 3, 2, 4)
            .reshape(B, NQ, D)
            for c in range(N_CORES)
        ],
        axis=0,
    )
    if _trace:
        return out, res
    return out
